# revision 18
# baseline (speedup 1.0000x reference)
# Trainium2 Bass kernel for a 2-layer relational GCN (R-GCN style message
# passing), SPMD across 8 NeuronCores.
#
# Formulation (per layer):
#   agg[v]  = sum_r ( sum_{e: dst=v, etype=r} h[src_e] ) @ W_r
#   out     = relu?( agg + h @ loop_w + b )
#
# Strategy:
#   * Destination nodes are sharded across the 8 cores (12500 each), grouped
#     into windows of 128. Edges are grouped by (window, relation, src-chunk)
#     into 128-edge tiles; the tile schedule (tile counts per group) is shared
#     by all cores (SPMD) while the work is defined by per-core index data.
#   * Source rows are fetched with the bulk GPSIMD dma_gather (int16 indices,
#     so the feature table is addressed through 4 chunk views of <=32k rows).
#     One gather call covers a whole (megagroup of windows) x chunk range.
#   * Per edge tile: one-hot matmul A_r^T += H^T(contract edges) x S where
#     S[e, v] = (dstpos[e] == v) is built on the Vector engine with a single
#     tensor_scalar(is_equal) against an iota row. Pad slots carry
#     dstpos=999 so they match no column and contribute nothing.
#     A_r^T accumulates in PSUM ([Din, 128] per relation, 2 banks/window).
#   * Per window: copy A^T (8 relations) PSUM->SBUF (cast fp16), then 9
#     matmuls agg^T += W_r^T @ A_r^T (+ loop_w^T @ h^T_window), bias + (relu)
#     on the Scalar engine, PE transpose to row layout, batched stores.
#   * Between layers: AllGather of the per-core h1 shards rebuilds the global
#     gather table.
#
# kernel() takes FULL unsharded inputs and returns the FULL output.

import math
import os

import numpy as np

P = 128          # partitions / edge-tile size / dst-window size
D = 128          # feature dim
NCORES = 8
NCHUNK = 4       # gather table chunks (int16 index limit)
MGW = 6          # windows per megagroup (gather call batching)
STORE_BATCH = 4  # windows per output-row store batch

_cache = {}


# ----------------------------------------------------------------------------
# Host-side scheduling
# ----------------------------------------------------------------------------

def _schedule(src, dst, etype, n_nodes, n_rel):
    """Uniform (window, relation, chunk) tile schedule + per-core indices.

    Layer 1 gathers from table1 (feats, node-id order); layer 2 from table2
    (allgathered h1 shards, core-major slot order). Chunk assignment of an
    edge differs per layer, so each layer has its own tile schedule and
    index/dstloc arrays.
    """
    E = src.shape[0]
    nc_nodes = n_nodes // NCORES
    nwin = math.ceil(nc_nodes / P)
    chunk_rows = math.ceil(n_nodes / NCHUNK)
    assert chunk_rows <= 32767

    core = dst // nc_nodes
    dl = dst - core * nc_nodes          # local dst == slot (identity layout)
    ewin = dl // P
    epos = dl - ewin * P

    # table rows per layer
    row1 = src                                   # table1: node-id order
    row2 = src                                   # table2: same (identity slots)
    # (with identity slot assignment, table2 row = src_core*nc_nodes +
    #  src_local = src)

    layers = []
    for row in (row1, row2):
        chunk = row // chunk_rows
        local = (row - chunk * chunk_rows).astype(np.int16)

        gid = ((core * nwin + ewin) * n_rel + etype) * NCHUNK + chunk
        counts = np.bincount(
            gid, minlength=NCORES * nwin * n_rel * NCHUNK
        ).reshape(NCORES, nwin, n_rel, NCHUNK)
        ktl = -(-counts.max(axis=0) // P)         # [nwin, n_rel, NCHUNK]
        # each (w, r) needs >= 1 tile so its PSUM region gets reset
        empty = ktl.sum(axis=2) == 0
        ktl[:, :, 0][empty] = 1

        T = int(ktl.sum())

        # tile ids in (megagroup, chunk, window, relation)-major order, so
        # each (megagroup, chunk) gather call covers a contiguous id range
        wi, ri, ci = np.meshgrid(
            np.arange(nwin), np.arange(n_rel), np.arange(NCHUNK), indexing="ij"
        )
        order = np.lexsort(
            (ri.ravel(), wi.ravel(), ci.ravel(), (wi // MGW).ravel())
        )  # flat group ids sorted by (mg, c, w, r)
        flat_ktl = ktl.reshape(-1)
        tid_flat = np.zeros(flat_ktl.shape[0], dtype=np.int64)
        tid_flat[order] = np.concatenate(
            [[0], np.cumsum(flat_ktl[order])[:-1]]
        )
        tid = tid_flat.reshape(nwin, n_rel, NCHUNK)

        idxw = np.zeros((NCORES, 128, (T * P) // 16), dtype=np.int16)
        dstloc = np.full((NCORES, P, T), 999.0, dtype=np.float32)

        for c in range(NCORES):
            es = np.flatnonzero(core == c)
            g = ((ewin[es] * n_rel + etype[es]) * NCHUNK + chunk[es]).astype(
                np.int64
            )
            o = np.argsort(g, kind="stable")
            es = es[o]
            g = g[o]
            gstart = np.searchsorted(g, np.arange(nwin * n_rel * NCHUNK))
            pos_in_group = np.arange(es.shape[0]) - gstart[g]
            slot = tid.reshape(-1)[g] * P + pos_in_group
            t = slot // P
            p = slot - t * P
            # pads: chunk-local row 0 (real row, killed by dstloc=999)
            flat = np.zeros(T * P, dtype=np.int16)
            flat[slot] = local[es]
            # wrapped-16 layout, replicated over the 8 Q7 core groups
            w16 = flat.reshape(-1, 16).T            # [16, T*P/16]
            idxw[c] = np.tile(w16, (8, 1))
            dstloc[c, p, t] = epos[es].astype(np.float32)

        layers.append(dict(ktl=ktl, T=T, tid=tid, idxw=idxw, dstloc=dstloc))

    return dict(nc_nodes=nc_nodes, nwin=nwin, chunk_rows=chunk_rows,
                layers=layers)


# ----------------------------------------------------------------------------
# Device program
# ----------------------------------------------------------------------------

def _build_program(n_nodes, n_rel, sched):
    import concourse.bass as bass
    import concourse.mybir as mybir
    import concourse.tile as tile
    from concourse import bacc
    from contextlib import ExitStack

    fp16 = mybir.dt.float16
    f32 = mybir.dt.float32
    i16 = mybir.dt.int16
    AF = mybir.ActivationFunctionType

    nc_nodes = sched["nc_nodes"]
    nwin = sched["nwin"]
    chunk_rows = sched["chunk_rows"]
    L = sched["layers"]
    Tmax = max(L[0]["T"], L[1]["T"])

    nc = bacc.Bacc(
        "TRN2",
        target_bir_lowering=False,
        debug=False,
        enable_asserts=False,
        num_devices=NCORES,
    )

    # ---- DRAM parameters ----
    table1 = nc.dram_tensor(
        "table1", [NCHUNK * chunk_rows, D], fp16, kind="ExternalInput"
    )
    featsT = nc.dram_tensor("featsT", [P, nwin * P], fp16, kind="ExternalInput")
    idx_d = [
        nc.dram_tensor(f"idxw{l}", [128, (L[l]["T"] * P) // 16], i16,
                       kind="ExternalInput")
        for l in range(2)
    ]
    dst_d = [
        nc.dram_tensor(f"dstloc{l}", [P, L[l]["T"]], f32, kind="ExternalInput")
        for l in range(2)
    ]
    w1_d = nc.dram_tensor("w1e", [P, (n_rel + 1) * D], fp16, kind="ExternalInput")
    w2_d = nc.dram_tensor("w2e", [P, (n_rel + 1) * D], fp16, kind="ExternalInput")
    b1_d = nc.dram_tensor("b1c", [P, 1], f32, kind="ExternalInput")
    b2_d = nc.dram_tensor("b2c", [P, 1], f32, kind="ExternalInput")
    iota_d = nc.dram_tensor("iota", [P, P], fp16, kind="ExternalInput")
    id16_d = nc.dram_tensor("id16", [P, P], fp16, kind="ExternalInput")
    id32_d = nc.dram_tensor("id32", [P, P], f32, kind="ExternalInput")

    out_d = nc.dram_tensor("out", [nc_nodes, D], f32, kind="ExternalOutput")

    h1shard = nc.dram_tensor("h1shard", [nc_nodes, D], fp16)
    table2 = nc.dram_tensor(
        "table2", [NCHUNK * chunk_rows, D], fp16, addr_space="Shared"
    )
    # table2 rows beyond n_nodes are never addressed by real indices; the
    # chunk views just need the space declared.

    # megagroups
    mgs = [
        list(range(m, min(m + MGW, nwin))) for m in range(0, nwin, MGW)
    ]
    # max tiles in one (mg, chunk) gather call (for pool sizing), per layer
    def call_tiles(l, mg, c):
        return int(L[l]["ktl"][mg, :, c].sum())

    max_call = max(
        call_tiles(l, mg, c)
        for l in range(2)
        for mg in [np.array(g) for g in mgs]
        for c in range(NCHUNK)
    )
    # max tiles in one megagroup across all chunks (idx slice sizing)
    max_mg = max(
        int(L[l]["ktl"][np.array(g)].sum()) for l in range(2) for g in mgs
    )

    with tile.TileContext(nc) as tc, ExitStack() as ctx:
        consts = ctx.enter_context(tc.tile_pool(name="consts", bufs=1))
        hbp = ctx.enter_context(tc.tile_pool(name="hb", bufs=5))
        idxp = ctx.enter_context(tc.tile_pool(name="idxmg", bufs=2))
        sp = ctx.enter_context(tc.tile_pool(name="sbuild", bufs=4))
        asbp = ctx.enter_context(tc.tile_pool(name="asb", bufs=2))
        htp = ctx.enter_context(tc.tile_pool(name="ht", bufs=2))
        rowp = ctx.enter_context(tc.tile_pool(name="rows", bufs=2))
        psA = ctx.enter_context(tc.tile_pool(name="psA", bufs=2, space="PSUM"))
        psG = ctx.enter_context(tc.tile_pool(name="psG", bufs=2, space="PSUM"))
        psT = ctx.enter_context(tc.tile_pool(name="psT", bufs=2, space="PSUM"))

        # ---- resident tiles ----
        w1sb = consts.tile([P, (n_rel + 1) * D], fp16, tag="w1")
        w2sb = consts.tile([P, (n_rel + 1) * D], fp16, tag="w2")
        iota = consts.tile([P, P], fp16, tag="iota")
        id16 = consts.tile([P, P], fp16, tag="id16")
        id32 = consts.tile([P, P], f32, tag="id32")
        b1sb = consts.tile([P, 1], f32, tag="b1")
        b2sb = consts.tile([P, 1], f32, tag="b2")
        dstloc_sb = [
            consts.tile([P, Tmax], f32, tag=f"dstloc{l}", name=f"dstloc{l}")
            for l in range(2)
        ]
        h1T_sb = consts.tile([P, nwin * P], fp16, tag="h1T")

        nc.sync.dma_start(out=w1sb[:], in_=w1_d[:])
        nc.sync.dma_start(out=w2sb[:], in_=w2_d[:])
        nc.sync.dma_start(out=iota[:], in_=iota_d[:])
        nc.sync.dma_start(out=id16[:], in_=id16_d[:])
        nc.sync.dma_start(out=id32[:], in_=id32_d[:])
        nc.sync.dma_start(out=b1sb[:], in_=b1_d[:])
        nc.sync.dma_start(out=b2sb[:], in_=b2_d[:])
        for l in range(2):
            nc.sync.dma_start(
                out=dstloc_sb[l][:, : L[l]["T"]], in_=dst_d[l][:]
            )

        def run_layer(layer):
            ktl = L[layer]["ktl"]
            tid = L[layer]["tid"]
            table = table1 if layer == 0 else table2
            wsb = w1sb if layer == 0 else w2sb
            bsb = b1sb if layer == 0 else b2sb
            dsb = dstloc_sb[layer]
            idxd = idx_d[layer]

            rows_tile = None
            rows_w0 = 0

            def flush_rows(w_end):
                nonlocal rows_tile, rows_w0
                if rows_tile is None:
                    return
                r0 = rows_w0 * P
                nvalid = min(w_end * P, nc_nodes) - r0
                dst_t = out_d if layer == 1 else h1shard
                full = nvalid // P
                if full > 0:
                    nc.sync.dma_start(
                        out=dst_t[r0 : r0 + full * P, :].rearrange(
                            "(j p) d -> p j d", p=P
                        ),
                        in_=rows_tile[:, : full * D].rearrange(
                            "p (j d) -> p j d", d=D
                        ),
                    )
                rem = nvalid - full * P
                if rem > 0:
                    nc.sync.dma_start(
                        out=dst_t[r0 + full * P : r0 + nvalid, :],
                        in_=rows_tile[:rem, full * D : full * D + D],
                    )
                rows_tile = None

            for mg in mgs:
                garr = np.array(mg)
                mg_tiles = int(ktl[garr].sum())
                # first tile id of this megagroup = min tid over its groups
                mg_t0 = int(tid[garr].min())

                # load this megagroup's gather indices (wrapped-16)
                idx_mg = idxp.tile(
                    [128, (max_mg * P) // 16], i16, tag="idxmg"
                )
                nc.sync.dma_start(
                    out=idx_mg[:, : (mg_tiles * P) // 16],
                    in_=idxd[:, (mg_t0 * P) // 16 : ((mg_t0 + mg_tiles) * P) // 16],
                )

                # one gather call per chunk; (mg, chunk) tile ids contiguous
                hb = {}
                seg0 = {}  # chunk -> first tile id of its call
                off = mg_t0
                for cph in range(NCHUNK):
                    ntc = call_tiles(layer, garr, cph)
                    if ntc == 0:
                        continue
                    hbt = hbp.tile([P, max_call * D], fp16, tag="hb")
                    nc.gpsimd.dma_gather(
                        out_ap=hbt[:, : ntc * D].rearrange(
                            "p (j d) -> p j d", d=D
                        ),
                        in_ap=table[
                            cph * chunk_rows : (cph + 1) * chunk_rows, :
                        ],
                        idxs_ap=idx_mg[
                            :,
                            ((off - mg_t0) * P) // 16 : ((off - mg_t0 + ntc) * P)
                            // 16,
                        ],
                        num_idxs=ntc * P,
                        num_idxs_reg=ntc * P,
                        elem_size=D,
                        single_packet=False,
                    )
                    hb[cph] = hbt
                    seg0[cph] = off
                    off += ntc

                for w in mg:
                    Apsum = psA.tile([P, n_rel * D], f32, tag="A", space="PSUM")
                    for r in range(n_rel):
                        seq = [
                            (cph, j)
                            for cph in range(NCHUNK)
                            for j in range(int(ktl[w, r, cph]))
                        ]
                        for si, (cph, j) in enumerate(seq):
                            t = int(tid[w, r, cph]) + j
                            col = t - seg0[cph]
                            S = sp.tile([P, P], fp16, tag="S")
                            nc.vector.tensor_scalar(
                                out=S[:],
                                in0=iota[:],
                                scalar1=dsb[:, t : t + 1],
                                scalar2=None,
                                op0=mybir.AluOpType.is_equal,
                            )
                            nc.tensor.matmul(
                                out=Apsum[:, r * D : (r + 1) * D],
                                lhsT=hb[cph][:, col * D : (col + 1) * D],
                                rhs=S[:],
                                start=(si == 0),
                                stop=(si == len(seq) - 1),
                            )

                    # copy A^T PSUM -> SBUF as fp16 (2 halves)
                    Asb = asbp.tile([P, n_rel * D], fp16, tag="Asb")
                    half = (n_rel * D) // 2
                    nc.scalar.copy(out=Asb[:, :half], in_=Apsum[:, :half])
                    nc.scalar.copy(out=Asb[:, half:], in_=Apsum[:, half:])

                    # transform: agg^T = sum_r W_r^T A_r^T + loop_w^T h^T + b
                    agg = psG.tile([P, P], f32, tag="agg", space="PSUM")
                    if layer == 0:
                        hT = htp.tile([P, P], fp16, tag="hT")
                        nc.sync.dma_start(
                            out=hT[:], in_=featsT[:, w * P : (w + 1) * P]
                        )
                        hT_ap = hT[:]
                    else:
                        hT_ap = h1T_sb[:, w * P : (w + 1) * P]
                    nc.tensor.matmul(
                        out=agg[:],
                        lhsT=wsb[:, n_rel * D : (n_rel + 1) * D],
                        rhs=hT_ap,
                        start=True,
                        stop=False,
                    )
                    for r in range(n_rel):
                        nc.tensor.matmul(
                            out=agg[:],
                            lhsT=wsb[:, r * D : (r + 1) * D],
                            rhs=Asb[:, r * D : (r + 1) * D],
                            start=False,
                            stop=(r == n_rel - 1),
                        )

                    # epilogue
                    if w % STORE_BATCH == 0:
                        flush_rows(w)
                        rows_w0 = w
                    if layer == 0:
                        nc.scalar.activation(
                            out=h1T_sb[:, w * P : (w + 1) * P],
                            in_=agg[:],
                            func=AF.Relu,
                            bias=bsb[:],
                        )
                        trp = psT.tile([P, P], fp16, tag="tr", space="PSUM")
                        nc.tensor.transpose(
                            out=trp[:],
                            in_=h1T_sb[:, w * P : (w + 1) * P],
                            identity=id16[:],
                        )
                        if rows_tile is None:
                            rows_tile = rowp.tile(
                                [P, STORE_BATCH * D], fp16, tag="rows16"
                            )
                        j = w - rows_w0
                        nc.vector.tensor_copy(
                            out=rows_tile[:, j * D : (j + 1) * D], in_=trp[:]
                        )
                    else:
                        oT = htp.tile([P, P], f32, tag="oT")
                        nc.scalar.activation(
                            out=oT[:], in_=agg[:], func=AF.Identity, bias=bsb[:]
                        )
                        trp = psT.tile([P, P], f32, tag="tr", space="PSUM")
                        nc.tensor.transpose(
                            out=trp[:], in_=oT[:], identity=id32[:]
                        )
                        if rows_tile is None:
                            rows_tile = rowp.tile(
                                [P, STORE_BATCH * D], f32, tag="rows32"
                            )
                        j = w - rows_w0
                        nc.vector.tensor_copy(
                            out=rows_tile[:, j * D : (j + 1) * D], in_=trp[:]
                        )
            flush_rows(nwin)

        run_layer(0)
        nc.gpsimd.collective_compute(
            "AllGather",
            mybir.AluOpType.bypass,
            replica_groups=[list(range(NCORES))],
            ins=[h1shard[:]],
            outs=[table2[:n_nodes, :]],
        )
        run_layer(1)

    nc.compile()
    return nc


# ----------------------------------------------------------------------------
# Entry point
# ----------------------------------------------------------------------------

def _plan(feats, W1, loop_w1, b1, W2, loop_w2, b2, src, dst, etype):
    """Returns (prog, in_maps, assemble) for the given full inputs."""
    feats = np.asarray(feats, dtype=np.float32)
    W1 = np.asarray(W1, dtype=np.float32)
    loop_w1 = np.asarray(loop_w1, dtype=np.float32)
    b1 = np.asarray(b1, dtype=np.float32)
    W2 = np.asarray(W2, dtype=np.float32)
    loop_w2 = np.asarray(loop_w2, dtype=np.float32)
    b2 = np.asarray(b2, dtype=np.float32)
    src = np.asarray(src, dtype=np.int64)
    dst = np.asarray(dst, dtype=np.int64)
    etype = np.asarray(etype, dtype=np.int64)

    n_nodes, d = feats.shape
    n_rel = W1.shape[0]
    assert d == D and n_nodes % NCORES == 0

    key = (n_nodes, src.shape[0], n_rel)
    if key not in _cache:
        sched = _schedule(src, dst, etype, n_nodes, n_rel)
        prog = _build_program(n_nodes, n_rel, sched)
        _cache[key] = (sched, prog)
    sched, prog = _cache[key]

    nc_nodes = sched["nc_nodes"]
    nwin = sched["nwin"]
    chunk_rows = sched["chunk_rows"]
    L = sched["layers"]

    # ---- build input maps ----
    table1 = np.zeros((NCHUNK * chunk_rows, D), dtype=np.float16)
    table1[:n_nodes] = feats.astype(np.float16)

    w1e = np.concatenate([W1, loop_w1[None]], axis=0).astype(np.float16)
    w1e = w1e.transpose(1, 0, 2).reshape(P, (n_rel + 1) * D).copy()
    w2e = np.concatenate([W2, loop_w2[None]], axis=0).astype(np.float16)
    w2e = w2e.transpose(1, 0, 2).reshape(P, (n_rel + 1) * D).copy()
    b1c = np.ascontiguousarray(b1.reshape(P, 1), dtype=np.float32)
    b2c = np.ascontiguousarray(b2.reshape(P, 1), dtype=np.float32)
    iota = np.broadcast_to(np.arange(P, dtype=np.float16), (P, P)).copy()
    id16 = np.eye(P, dtype=np.float16)
    id32 = np.eye(P, dtype=np.float32)

    in_maps = []
    for c in range(NCORES):
        fT = np.zeros((P, nwin * P), dtype=np.float16)
        fT[:, :nc_nodes] = feats[c * nc_nodes : (c + 1) * nc_nodes].astype(
            np.float16
        ).T
        in_maps.append(
            dict(
                table1=table1,
                featsT=fT,
                idxw0=L[0]["idxw"][c],
                idxw1=L[1]["idxw"][c],
                dstloc0=L[0]["dstloc"][c],
                dstloc1=L[1]["dstloc"][c],
                w1e=w1e,
                w2e=w2e,
                b1c=b1c,
                b2c=b2c,
                iota=iota,
                id16=id16,
                id32=id32,
            )
        )

    def assemble(shards):
        out = np.zeros((n_nodes, D), dtype=np.float32)
        for c in range(NCORES):
            out[c * nc_nodes : (c + 1) * nc_nodes] = shards[c]
        return out

    return prog, in_maps, assemble


def kernel(feats, W1, loop_w1, b1, W2, loop_w2, b2, src, dst, etype):
    prog, in_maps, assemble = _plan(
        feats, W1, loop_w1, b1, W2, loop_w2, b2, src, dst, etype
    )
    from concourse.bass_utils import run_bass_kernel_spmd

    trace = os.environ.get("BASS_KERNEL_TRACE", "0") == "1"
    res = run_bass_kernel_spmd(prog, in_maps, list(range(NCORES)), trace=trace)
    global _last_exec_ns
    _last_exec_ns = res.exec_time_ns

    return assemble([res.results[c]["out"] for c in range(NCORES)])


_last_exec_ns = None


# revision 19
# speedup vs baseline: 1.0386x; 1.0386x over previous
# Trainium2 Bass kernel for a 2-layer relational GCN (R-GCN style message
# passing), SPMD across 8 NeuronCores.
#
# Formulation (per layer):
#   agg[v]  = sum_r ( sum_{e: dst=v, etype=r} h[src_e] ) @ W_r
#   out     = relu?( agg + h @ loop_w + b )
#
# Strategy:
#   * Destination nodes are sharded across the 8 cores (12500 each), grouped
#     into windows of 128. Edges are grouped by (window, relation, src-chunk)
#     into 128-edge tiles; the tile schedule (tile counts per group) is shared
#     by all cores (SPMD) while the work is defined by per-core index data.
#   * Source rows are fetched with the bulk GPSIMD dma_gather (int16 indices,
#     so the feature table is addressed through 4 chunk views of <=32k rows).
#     One gather call covers a whole (megagroup of windows) x chunk range.
#   * Per edge tile: one-hot matmul A_r^T += H^T(contract edges) x S where
#     S[e, v] = (dstpos[e] == v) is built on the Vector engine with a single
#     tensor_scalar(is_equal) against an iota row. Pad slots carry
#     dstpos=999 so they match no column and contribute nothing.
#     A_r^T accumulates in PSUM ([Din, 128] per relation, 2 banks/window).
#   * Per window: copy A^T (8 relations) PSUM->SBUF (cast fp16), then 9
#     matmuls agg^T += W_r^T @ A_r^T (+ loop_w^T @ h^T_window), bias + (relu)
#     on the Scalar engine, PE transpose to row layout, batched stores.
#   * Between layers: AllGather of the per-core h1 shards rebuilds the global
#     gather table.
#
# kernel() takes FULL unsharded inputs and returns the FULL output.

import math
import os

import numpy as np

P = 128          # partitions / edge-tile size / dst-window size
D = 128          # feature dim
NCORES = 8
NCHUNK = 4       # gather table chunks (int16 index limit)
MGW = 6          # windows per megagroup (gather call batching)
STORE_BATCH = 4  # windows per output-row store batch

_cache = {}


# ----------------------------------------------------------------------------
# Host-side scheduling
# ----------------------------------------------------------------------------

def _schedule(src, dst, etype, n_nodes, n_rel):
    """Uniform (window, relation, chunk) tile schedule + per-core indices.

    Layer 1 gathers from table1 (feats, node-id order); layer 2 from table2
    (allgathered h1 shards, core-major slot order). Chunk assignment of an
    edge differs per layer, so each layer has its own tile schedule and
    index/dstloc arrays.
    """
    E = src.shape[0]
    nc_nodes = n_nodes // NCORES
    nwin = math.ceil(nc_nodes / P)
    chunk_rows = math.ceil(n_nodes / NCHUNK)
    assert chunk_rows <= 32767

    core = dst // nc_nodes
    dl = dst - core * nc_nodes          # local dst == slot (identity layout)
    ewin = dl // P
    epos = dl - ewin * P

    # table rows per layer
    row1 = src                                   # table1: node-id order
    row2 = src                                   # table2: same (identity slots)
    # (with identity slot assignment, table2 row = src_core*nc_nodes +
    #  src_local = src)

    layers = []
    for row in (row1, row2):
        chunk = row // chunk_rows
        local = (row - chunk * chunk_rows).astype(np.int16)

        gid = ((core * nwin + ewin) * n_rel + etype) * NCHUNK + chunk
        counts = np.bincount(
            gid, minlength=NCORES * nwin * n_rel * NCHUNK
        ).reshape(NCORES, nwin, n_rel, NCHUNK)
        ktl = -(-counts.max(axis=0) // P)         # [nwin, n_rel, NCHUNK]
        # each (w, r) needs >= 1 tile so its PSUM region gets reset
        empty = ktl.sum(axis=2) == 0
        ktl[:, :, 0][empty] = 1

        T = int(ktl.sum())

        # tile ids in (megagroup, chunk, window, relation)-major order, so
        # each (megagroup, chunk) gather call covers a contiguous id range
        wi, ri, ci = np.meshgrid(
            np.arange(nwin), np.arange(n_rel), np.arange(NCHUNK), indexing="ij"
        )
        order = np.lexsort(
            (ri.ravel(), wi.ravel(), ci.ravel(), (wi // MGW).ravel())
        )  # flat group ids sorted by (mg, c, w, r)
        flat_ktl = ktl.reshape(-1)
        tid_flat = np.zeros(flat_ktl.shape[0], dtype=np.int64)
        tid_flat[order] = np.concatenate(
            [[0], np.cumsum(flat_ktl[order])[:-1]]
        )
        tid = tid_flat.reshape(nwin, n_rel, NCHUNK)

        idxw = np.zeros((NCORES, 128, (T * P) // 16), dtype=np.int16)
        dstloc = np.full((NCORES, P, T), 999.0, dtype=np.float32)

        for c in range(NCORES):
            es = np.flatnonzero(core == c)
            g = ((ewin[es] * n_rel + etype[es]) * NCHUNK + chunk[es]).astype(
                np.int64
            )
            o = np.argsort(g, kind="stable")
            es = es[o]
            g = g[o]
            gstart = np.searchsorted(g, np.arange(nwin * n_rel * NCHUNK))
            pos_in_group = np.arange(es.shape[0]) - gstart[g]
            slot = tid.reshape(-1)[g] * P + pos_in_group
            t = slot // P
            p = slot - t * P
            # pads: chunk-local row 0 (real row, killed by dstloc=999)
            flat = np.zeros(T * P, dtype=np.int16)
            flat[slot] = local[es]
            # wrapped-16 layout, replicated over the 8 Q7 core groups
            w16 = flat.reshape(-1, 16).T            # [16, T*P/16]
            idxw[c] = np.tile(w16, (8, 1))
            dstloc[c, p, t] = epos[es].astype(np.float32)

        layers.append(dict(ktl=ktl, T=T, tid=tid, idxw=idxw, dstloc=dstloc))

    return dict(nc_nodes=nc_nodes, nwin=nwin, chunk_rows=chunk_rows,
                layers=layers)


# ----------------------------------------------------------------------------
# Device program
# ----------------------------------------------------------------------------

def _build_program(n_nodes, n_rel, sched):
    import concourse.bass as bass
    import concourse.mybir as mybir
    import concourse.tile as tile
    from concourse import bacc
    from contextlib import ExitStack

    fp16 = mybir.dt.float16
    f32 = mybir.dt.float32
    i16 = mybir.dt.int16
    AF = mybir.ActivationFunctionType

    nc_nodes = sched["nc_nodes"]
    nwin = sched["nwin"]
    chunk_rows = sched["chunk_rows"]
    L = sched["layers"]
    Tmax = max(L[0]["T"], L[1]["T"])

    nc = bacc.Bacc(
        "TRN2",
        target_bir_lowering=False,
        debug=False,
        enable_asserts=False,
        num_devices=NCORES,
    )

    # ---- DRAM parameters ----
    table1 = nc.dram_tensor(
        "table1", [NCHUNK * chunk_rows, D], fp16, kind="ExternalInput"
    )
    featsT = nc.dram_tensor("featsT", [P, nwin * P], fp16, kind="ExternalInput")
    idx_d = [
        nc.dram_tensor(f"idxw{l}", [128, (L[l]["T"] * P) // 16], i16,
                       kind="ExternalInput")
        for l in range(2)
    ]
    dst_d = [
        nc.dram_tensor(f"dstloc{l}", [P, L[l]["T"]], f32, kind="ExternalInput")
        for l in range(2)
    ]
    w1_d = nc.dram_tensor("w1e", [P, (n_rel + 1) * D], fp16, kind="ExternalInput")
    w2_d = nc.dram_tensor("w2e", [P, (n_rel + 1) * D], fp16, kind="ExternalInput")
    b1_d = nc.dram_tensor("b1c", [P, 1], f32, kind="ExternalInput")
    b2_d = nc.dram_tensor("b2c", [P, 1], f32, kind="ExternalInput")
    iota_d = nc.dram_tensor("iota", [P, P], fp16, kind="ExternalInput")
    id16_d = nc.dram_tensor("id16", [P, P], fp16, kind="ExternalInput")
    id32_d = nc.dram_tensor("id32", [P, P], f32, kind="ExternalInput")

    out_d = nc.dram_tensor("out", [nc_nodes, D], f32, kind="ExternalOutput")

    h1shard = nc.dram_tensor("h1shard", [nc_nodes, D], fp16)
    table2 = nc.dram_tensor(
        "table2", [NCHUNK * chunk_rows, D], fp16, addr_space="Shared"
    )
    # table2 rows beyond n_nodes are never addressed by real indices; the
    # chunk views just need the space declared.

    # megagroups
    mgs = [
        list(range(m, min(m + MGW, nwin))) for m in range(0, nwin, MGW)
    ]
    # max tiles in one (mg, chunk) gather call (for pool sizing), per layer
    def call_tiles(l, mg, c):
        return int(L[l]["ktl"][mg, :, c].sum())

    max_call = max(
        call_tiles(l, mg, c)
        for l in range(2)
        for mg in [np.array(g) for g in mgs]
        for c in range(NCHUNK)
    )
    # max tiles in one megagroup across all chunks (idx slice sizing)
    max_mg = max(
        int(L[l]["ktl"][np.array(g)].sum()) for l in range(2) for g in mgs
    )

    with tile.TileContext(nc) as tc, ExitStack() as ctx:
        consts = ctx.enter_context(tc.tile_pool(name="consts", bufs=1))
        hbp = ctx.enter_context(tc.tile_pool(name="hb", bufs=5))
        idxp = ctx.enter_context(tc.tile_pool(name="idxmg", bufs=2))
        sp = ctx.enter_context(tc.tile_pool(name="sbuild", bufs=4))
        asbp = ctx.enter_context(tc.tile_pool(name="asb", bufs=2))
        htp = ctx.enter_context(tc.tile_pool(name="ht", bufs=2))
        rowp = ctx.enter_context(tc.tile_pool(name="rows", bufs=2))
        psA = ctx.enter_context(tc.tile_pool(name="psA", bufs=2, space="PSUM"))
        psG = ctx.enter_context(tc.tile_pool(name="psG", bufs=2, space="PSUM"))
        psT = ctx.enter_context(tc.tile_pool(name="psT", bufs=2, space="PSUM"))

        # ---- resident tiles ----
        w1sb = consts.tile([P, (n_rel + 1) * D], fp16, tag="w1")
        w2sb = consts.tile([P, (n_rel + 1) * D], fp16, tag="w2")
        iota = consts.tile([P, P], fp16, tag="iota")
        id16 = consts.tile([P, P], fp16, tag="id16")
        id32 = consts.tile([P, P], f32, tag="id32")
        b1sb = consts.tile([P, 1], f32, tag="b1")
        b2sb = consts.tile([P, 1], f32, tag="b2")
        dstloc_sb = [
            consts.tile([P, Tmax], f32, tag=f"dstloc{l}", name=f"dstloc{l}")
            for l in range(2)
        ]
        h1T_sb = consts.tile([P, nwin * P], fp16, tag="h1T")

        nc.sync.dma_start(out=w1sb[:], in_=w1_d[:])
        nc.sync.dma_start(out=w2sb[:], in_=w2_d[:])
        nc.sync.dma_start(out=iota[:], in_=iota_d[:])
        nc.sync.dma_start(out=id16[:], in_=id16_d[:])
        nc.sync.dma_start(out=id32[:], in_=id32_d[:])
        nc.sync.dma_start(out=b1sb[:], in_=b1_d[:])
        nc.sync.dma_start(out=b2sb[:], in_=b2_d[:])
        for l in range(2):
            nc.sync.dma_start(
                out=dstloc_sb[l][:, : L[l]["T"]], in_=dst_d[l][:]
            )

        def run_layer(layer):
            ktl = L[layer]["ktl"]
            tid = L[layer]["tid"]
            table = table1 if layer == 0 else table2
            wsb = w1sb if layer == 0 else w2sb
            bsb = b1sb if layer == 0 else b2sb
            dsb = dstloc_sb[layer]
            idxd = idx_d[layer]

            rows_tile = None
            rows_w0 = 0

            def flush_rows(w_end):
                nonlocal rows_tile, rows_w0
                if rows_tile is None:
                    return
                r0 = rows_w0 * P
                nvalid = min(w_end * P, nc_nodes) - r0
                dst_t = out_d if layer == 1 else h1shard
                full = nvalid // P
                if full > 0:
                    nc.sync.dma_start(
                        out=dst_t[r0 : r0 + full * P, :].rearrange(
                            "(j p) d -> p j d", p=P
                        ),
                        in_=rows_tile[:, : full * D].rearrange(
                            "p (j d) -> p j d", d=D
                        ),
                    )
                rem = nvalid - full * P
                if rem > 0:
                    nc.sync.dma_start(
                        out=dst_t[r0 + full * P : r0 + nvalid, :],
                        in_=rows_tile[:rem, full * D : full * D + D],
                    )
                rows_tile = None

            for mg in mgs:
                garr = np.array(mg)
                mg_tiles = int(ktl[garr].sum())
                # first tile id of this megagroup = min tid over its groups
                mg_t0 = int(tid[garr].min())

                # load this megagroup's gather indices (wrapped-16)
                idx_mg = idxp.tile(
                    [128, (max_mg * P) // 16], i16, tag="idxmg"
                )
                nc.sync.dma_start(
                    out=idx_mg[:, : (mg_tiles * P) // 16],
                    in_=idxd[:, (mg_t0 * P) // 16 : ((mg_t0 + mg_tiles) * P) // 16],
                )

                # one gather call per chunk; (mg, chunk) tile ids contiguous
                hb = {}
                seg0 = {}  # chunk -> first tile id of its call
                off = mg_t0
                for cph in range(NCHUNK):
                    ntc = call_tiles(layer, garr, cph)
                    if ntc == 0:
                        continue
                    hbt = hbp.tile([P, max_call * D], fp16, tag="hb")
                    nc.gpsimd.dma_gather(
                        out_ap=hbt[:, : ntc * D].rearrange(
                            "p (j d) -> p j d", d=D
                        ),
                        in_ap=table[
                            cph * chunk_rows : (cph + 1) * chunk_rows, :
                        ],
                        idxs_ap=idx_mg[
                            :,
                            ((off - mg_t0) * P) // 16 : ((off - mg_t0 + ntc) * P)
                            // 16,
                        ],
                        num_idxs=ntc * P,
                        num_idxs_reg=ntc * P,
                        elem_size=D,
                        single_packet=False,
                    )
                    hb[cph] = hbt
                    seg0[cph] = off
                    off += ntc

                for w in mg:
                    Apsum = psA.tile([P, n_rel * D], f32, tag="A", space="PSUM")
                    for r in range(n_rel):
                        seq = [
                            (cph, j)
                            for cph in range(NCHUNK)
                            for j in range(int(ktl[w, r, cph]))
                        ]
                        for si, (cph, j) in enumerate(seq):
                            t = int(tid[w, r, cph]) + j
                            col = t - seg0[cph]
                            S = sp.tile([P, P], fp16, tag="S")
                            nc.vector.tensor_scalar(
                                out=S[:],
                                in0=iota[:],
                                scalar1=dsb[:, t : t + 1],
                                scalar2=None,
                                op0=mybir.AluOpType.is_equal,
                            )
                            nc.tensor.matmul(
                                out=Apsum[:, r * D : (r + 1) * D],
                                lhsT=hb[cph][:, col * D : (col + 1) * D],
                                rhs=S[:],
                                start=(si == 0),
                                stop=(si == len(seq) - 1),
                            )

                    # copy A^T PSUM -> SBUF as fp16 (2 halves)
                    Asb = asbp.tile([P, n_rel * D], fp16, tag="Asb")
                    half = (n_rel * D) // 2
                    nc.scalar.copy(out=Asb[:, :half], in_=Apsum[:, :half])
                    nc.scalar.copy(out=Asb[:, half:], in_=Apsum[:, half:])

                    # transform: agg^T = sum_r W_r^T A_r^T + loop_w^T h^T + b
                    agg = psG.tile([P, P], f32, tag="agg", space="PSUM")
                    if layer == 0:
                        hT = htp.tile([P, P], fp16, tag="hT")
                        nc.sync.dma_start(
                            out=hT[:], in_=featsT[:, w * P : (w + 1) * P]
                        )
                        hT_ap = hT[:]
                    else:
                        hT_ap = h1T_sb[:, w * P : (w + 1) * P]
                    nc.tensor.matmul(
                        out=agg[:],
                        lhsT=wsb[:, n_rel * D : (n_rel + 1) * D],
                        rhs=hT_ap,
                        start=True,
                        stop=False,
                    )
                    for r in range(n_rel):
                        nc.tensor.matmul(
                            out=agg[:],
                            lhsT=wsb[:, r * D : (r + 1) * D],
                            rhs=Asb[:, r * D : (r + 1) * D],
                            start=False,
                            stop=(r == n_rel - 1),
                        )

                    # epilogue
                    if w % STORE_BATCH == 0:
                        flush_rows(w)
                        rows_w0 = w
                    if layer == 0:
                        nc.scalar.activation(
                            out=h1T_sb[:, w * P : (w + 1) * P],
                            in_=agg[:],
                            func=AF.Relu,
                            bias=bsb[:],
                        )
                        trp = psT.tile([P, P], fp16, tag="tr", space="PSUM")
                        nc.tensor.transpose(
                            out=trp[:],
                            in_=h1T_sb[:, w * P : (w + 1) * P],
                            identity=id16[:],
                        )
                        if rows_tile is None:
                            rows_tile = rowp.tile(
                                [P, STORE_BATCH * D], fp16, tag="rows16"
                            )
                        j = w - rows_w0
                        nc.vector.tensor_copy(
                            out=rows_tile[:, j * D : (j + 1) * D], in_=trp[:]
                        )
                    else:
                        oT = htp.tile([P, P], f32, tag="oT")
                        nc.scalar.activation(
                            out=oT[:], in_=agg[:], func=AF.Identity, bias=bsb[:]
                        )
                        trp = psT.tile([P, P], f32, tag="tr", space="PSUM")
                        nc.tensor.transpose(
                            out=trp[:], in_=oT[:], identity=id32[:]
                        )
                        if rows_tile is None:
                            rows_tile = rowp.tile(
                                [P, STORE_BATCH * D], f32, tag="rows32"
                            )
                        j = w - rows_w0
                        nc.vector.tensor_copy(
                            out=rows_tile[:, j * D : (j + 1) * D], in_=trp[:]
                        )
            flush_rows(nwin)

        repeat = int(os.environ.get("KERNEL_REPEAT", "1"))
        for _rep in range(repeat):
            run_layer(0)
            nc.gpsimd.collective_compute(
                "AllGather",
                mybir.AluOpType.bypass,
                replica_groups=[list(range(NCORES))],
                ins=[h1shard[:]],
                outs=[table2[:n_nodes, :]],
            )
            run_layer(1)

    nc.compile()
    return nc


# ----------------------------------------------------------------------------
# Entry point
# ----------------------------------------------------------------------------

def _plan(feats, W1, loop_w1, b1, W2, loop_w2, b2, src, dst, etype):
    """Returns (prog, in_maps, assemble) for the given full inputs."""
    feats = np.asarray(feats, dtype=np.float32)
    W1 = np.asarray(W1, dtype=np.float32)
    loop_w1 = np.asarray(loop_w1, dtype=np.float32)
    b1 = np.asarray(b1, dtype=np.float32)
    W2 = np.asarray(W2, dtype=np.float32)
    loop_w2 = np.asarray(loop_w2, dtype=np.float32)
    b2 = np.asarray(b2, dtype=np.float32)
    src = np.asarray(src, dtype=np.int64)
    dst = np.asarray(dst, dtype=np.int64)
    etype = np.asarray(etype, dtype=np.int64)

    n_nodes, d = feats.shape
    n_rel = W1.shape[0]
    assert d == D and n_nodes % NCORES == 0

    key = (n_nodes, src.shape[0], n_rel)
    if key not in _cache:
        sched = _schedule(src, dst, etype, n_nodes, n_rel)
        prog = _build_program(n_nodes, n_rel, sched)
        _cache[key] = (sched, prog)
    sched, prog = _cache[key]

    nc_nodes = sched["nc_nodes"]
    nwin = sched["nwin"]
    chunk_rows = sched["chunk_rows"]
    L = sched["layers"]

    # ---- build input maps ----
    table1 = np.zeros((NCHUNK * chunk_rows, D), dtype=np.float16)
    table1[:n_nodes] = feats.astype(np.float16)

    w1e = np.concatenate([W1, loop_w1[None]], axis=0).astype(np.float16)
    w1e = w1e.transpose(1, 0, 2).reshape(P, (n_rel + 1) * D).copy()
    w2e = np.concatenate([W2, loop_w2[None]], axis=0).astype(np.float16)
    w2e = w2e.transpose(1, 0, 2).reshape(P, (n_rel + 1) * D).copy()
    b1c = np.ascontiguousarray(b1.reshape(P, 1), dtype=np.float32)
    b2c = np.ascontiguousarray(b2.reshape(P, 1), dtype=np.float32)
    iota = np.broadcast_to(np.arange(P, dtype=np.float16), (P, P)).copy()
    id16 = np.eye(P, dtype=np.float16)
    id32 = np.eye(P, dtype=np.float32)

    in_maps = []
    for c in range(NCORES):
        fT = np.zeros((P, nwin * P), dtype=np.float16)
        fT[:, :nc_nodes] = feats[c * nc_nodes : (c + 1) * nc_nodes].astype(
            np.float16
        ).T
        in_maps.append(
            dict(
                table1=table1,
                featsT=fT,
                idxw0=L[0]["idxw"][c],
                idxw1=L[1]["idxw"][c],
                dstloc0=L[0]["dstloc"][c],
                dstloc1=L[1]["dstloc"][c],
                w1e=w1e,
                w2e=w2e,
                b1c=b1c,
                b2c=b2c,
                iota=iota,
                id16=id16,
                id32=id32,
            )
        )

    def assemble(shards):
        out = np.zeros((n_nodes, D), dtype=np.float32)
        for c in range(NCORES):
            out[c * nc_nodes : (c + 1) * nc_nodes] = shards[c]
        return out

    return prog, in_maps, assemble


def kernel(feats, W1, loop_w1, b1, W2, loop_w2, b2, src, dst, etype):
    prog, in_maps, assemble = _plan(
        feats, W1, loop_w1, b1, W2, loop_w2, b2, src, dst, etype
    )
    from concourse.bass_utils import run_bass_kernel_spmd

    trace = os.environ.get("BASS_KERNEL_TRACE", "0") == "1"
    res = run_bass_kernel_spmd(prog, in_maps, list(range(NCORES)), trace=trace)
    global _last_exec_ns
    _last_exec_ns = res.exec_time_ns

    return assemble([res.results[c]["out"] for c in range(NCORES)])


_last_exec_ns = None
